# revision 1
# baseline (speedup 1.0000x reference)
"""Dense MoE layer (8 experts, all-expert weighted combine) on 8 TRN2 NeuronCores.

Strategy: data-parallel over the token dim. Each core gets a 1024-token shard
(pre-transposed + bf16-cast on host), the full stacked expert weights (bf16),
and computes gate softmax + all 8 expert matmuls + gate-weighted combine
locally. No collectives; host concatenates the 8 output shards.

Per-core device schedule:
  - xT (bf16 [1024, 1024]) resident in SBUF, We[e] streamed per expert
    (double buffered).
  - gate: logits psum[n,8] via 8 K-chunk matmuls + a K=1 ones-row matmul to
    add bg; softmax on DVE/ACT; g transposed via PE so bias term g@be can be
    computed as a K=8 matmul that initializes the output accumulator.
  - main: per (expert, token-tile): 16 matmuls (8 K-chunks x 2 N=512 halves)
    accumulate y = x @ We[e] in PSUM f32; then one fused DVE op per half:
    out = psum * g[:, e] + out  (scalar_tensor_tensor, per-partition scalar).
  - out f32 [1024, 1024] DMAed back.
"""

import os
import sys

import numpy as np

try:
    import concourse.bass as bass  # noqa: F401
except ImportError:  # harness containers stage the repo at /opt/trn_rl_repo
    sys.path.insert(0, "/opt/trn_rl_repo")

from contextlib import ExitStack

import ml_dtypes

import concourse.bass as bass
import concourse.mybir as mybir
import concourse.tile as tile
from concourse import bacc
from concourse.bass_utils import run_bass_kernel_spmd
from concourse.masks import make_identity

N_CORES = 8
N_TOK = 8192
IN_F = 1024
OUT_F = 1024
E = 8
P = 128  # partitions


def build_nc(n_tok_pc: int = N_TOK // N_CORES, debug: bool = False):
    """Build the single-core SPMD Bass program (same program on all 8 cores)."""
    fp32 = mybir.dt.float32
    bf16 = mybir.dt.bfloat16

    K_CH = IN_F // P  # contraction chunks of 128
    T = n_tok_pc // P  # token tiles per core
    NH = OUT_F // 512  # N=512 halves per PSUM-bank-limited matmul

    nc = bacc.Bacc(
        "TRN2", target_bir_lowering=False, debug=debug, enable_asserts=False
    )

    xT = nc.declare_dram_parameter("xT", [IN_F, n_tok_pc], bf16, isOutput=False)
    We = nc.declare_dram_parameter("We", [E, IN_F, OUT_F], bf16, isOutput=False)
    be = nc.declare_dram_parameter("be", [E, OUT_F], bf16, isOutput=False)
    Wg = nc.declare_dram_parameter("Wg", [P, K_CH, E], bf16, isOutput=False)
    bg = nc.declare_dram_parameter("bg", [1, E], bf16, isOutput=False)
    out = nc.declare_dram_parameter("out", [n_tok_pc, OUT_F], fp32, isOutput=True)

    with tile.TileContext(nc) as tc, ExitStack() as ctx:
        consts = ctx.enter_context(tc.tile_pool(name="consts", bufs=1))
        xpool = ctx.enter_context(tc.tile_pool(name="xpool", bufs=1))
        wepool = ctx.enter_context(tc.tile_pool(name="wepool", bufs=2))
        opool = ctx.enter_context(tc.tile_pool(name="opool", bufs=1))
        gpool = ctx.enter_context(tc.tile_pool(name="gpool", bufs=1))
        small = ctx.enter_context(tc.tile_pool(name="small", bufs=4))
        psum_y = ctx.enter_context(tc.tile_pool(name="psum_y", bufs=2, space="PSUM"))
        psum_g = ctx.enter_context(tc.tile_pool(name="psum_g", bufs=2, space="PSUM"))
        psum_t = ctx.enter_context(tc.tile_pool(name="psum_t", bufs=1, space="PSUM"))

        # ---- constants / small inputs ----
        ident = consts.tile([P, P], fp32)
        make_identity(nc, ident)
        ones_row = consts.tile([1, P], bf16)
        nc.vector.memset(ones_row, 1.0)

        wg_sb = consts.tile([P, K_CH, E], bf16)
        nc.sync.dma_start(out=wg_sb, in_=Wg[:, :, :])
        bg_sb = consts.tile([1, E], bf16)
        nc.sync.dma_start(out=bg_sb, in_=bg[:, :])
        be_sb = consts.tile([E, OUT_F], bf16)
        nc.sync.dma_start(out=be_sb, in_=be[:, :])

        # ---- resident activations (pre-transposed shard) ----
        xT_sb = xpool.tile([P, K_CH, n_tok_pc], bf16)
        for c in range(K_CH):
            nc.sync.dma_start(out=xT_sb[:, c, :], in_=xT[c * P : (c + 1) * P, :])

        # resident gate weights + transposed gates + output accumulators
        g_sb = gpool.tile([P, T, E], fp32)
        gT_sb = gpool.tile([E, T, P], bf16)
        out_sb = opool.tile([P, T, OUT_F], fp32)

        # ---- gate phase: logits -> softmax -> transpose -> bias init ----
        for t in range(T):
            tok = slice(t * P, (t + 1) * P)

            lg = psum_g.tile([P, E], fp32, tag="lg")
            for c in range(K_CH):
                nc.tensor.matmul(
                    lg,
                    lhsT=xT_sb[:, c, tok],
                    rhs=wg_sb[:, c, :],
                    start=(c == 0),
                    stop=False,
                )
            # += 1 * bg  (K=1 matmul appends the gate bias)
            nc.tensor.matmul(lg, lhsT=ones_row, rhs=bg_sb, start=False, stop=True)

            # softmax over the 8 experts (free dim)
            neg_m = small.tile([P, 1], fp32, tag="negm")
            nc.vector.reduce_max(
                out=neg_m, in_=lg, axis=mybir.AxisListType.X, negate=True
            )
            gexp = small.tile([P, E], fp32, tag="gexp")
            nc.scalar.activation(
                out=gexp,
                in_=lg,
                func=mybir.ActivationFunctionType.Exp,
                bias=neg_m,
                scale=1.0,
            )
            ssum = small.tile([P, 1], fp32, tag="ssum")
            nc.vector.reduce_sum(out=ssum, in_=gexp, axis=mybir.AxisListType.X)
            rsum = small.tile([P, 1], fp32, tag="rsum")
            nc.vector.reciprocal(out=rsum, in_=ssum)
            nc.vector.tensor_scalar_mul(g_sb[:, t, :], gexp, rsum)

            # gT (bf16) so the bias term g @ be becomes a K=8 matmul
            gt_ps = psum_t.tile([E, P], fp32, tag="gt")
            nc.tensor.transpose(gt_ps, g_sb[:, t, :], ident)
            nc.scalar.copy(out=gT_sb[:, t, :], in_=gt_ps)

            # out_sb[t] = g @ be  (initializes the combine accumulator)
            pb = psum_y.tile([P, OUT_F], fp32, tag="y")
            for h in range(NH):
                hs = slice(h * 512, (h + 1) * 512)
                nc.tensor.matmul(
                    pb[:, hs], lhsT=gT_sb[:, t, :], rhs=be_sb[:, hs],
                    start=True, stop=True,
                )
            for h in range(NH):
                hs = slice(h * 512, (h + 1) * 512)
                nc.scalar.copy(out=out_sb[:, t, hs], in_=pb[:, hs])

        # ---- main phase: per expert, per token tile ----
        for e in range(E):
            we_sb = wepool.tile([P, K_CH, OUT_F], bf16, tag="we")
            for c in range(K_CH):
                nc.sync.dma_start(
                    out=we_sb[:, c, :], in_=We[e, c * P : (c + 1) * P, :]
                )
            for t in range(T):
                tok = slice(t * P, (t + 1) * P)
                py = psum_y.tile([P, OUT_F], fp32, tag="y")
                for c in range(K_CH):
                    for h in range(NH):
                        hs = slice(h * 512, (h + 1) * 512)
                        nc.tensor.matmul(
                            py[:, hs],
                            lhsT=xT_sb[:, c, tok],
                            rhs=we_sb[:, c, hs],
                            start=(c == 0),
                            stop=(c == K_CH - 1),
                        )
                # out += g[:, e] * y   (fused multiply-accumulate on DVE)
                for h in range(NH):
                    hs = slice(h * 512, (h + 1) * 512)
                    nc.vector.scalar_tensor_tensor(
                        out=out_sb[:, t, hs],
                        in0=py[:, hs],
                        scalar=g_sb[:, t, e : e + 1],
                        in1=out_sb[:, t, hs],
                        op0=mybir.AluOpType.mult,
                        op1=mybir.AluOpType.add,
                    )

        # ---- write back ----
        for t in range(T):
            nc.sync.dma_start(
                out=out[t * P : (t + 1) * P, :], in_=out_sb[:, t, :]
            )

    nc.compile()
    return nc


_NC_CACHE: dict = {}


def _get_nc(n_tok_pc: int):
    if n_tok_pc not in _NC_CACHE:
        _NC_CACHE[n_tok_pc] = build_nc(n_tok_pc)
    return _NC_CACHE[n_tok_pc]


def make_in_maps(x, We, be, Wg, bg):
    """Host-side sharding: token-shard + transpose x, bf16-cast everything."""
    bf16 = ml_dtypes.bfloat16
    x = np.asarray(x)
    n_tok_pc = x.shape[0] // N_CORES
    We_bf = np.asarray(We).astype(bf16)
    be_bf = np.asarray(be).astype(bf16)
    K_CH = IN_F // P
    # [1024, 8] -> [p, chunk, e]
    Wg_bf = (
        np.asarray(Wg).astype(bf16).reshape(K_CH, P, E).transpose(1, 0, 2).copy()
    )
    bg_bf = np.asarray(bg).astype(bf16).reshape(1, E)
    xbf = x.astype(bf16)
    in_maps = []
    for cid in range(N_CORES):
        xs = xbf[cid * n_tok_pc : (cid + 1) * n_tok_pc]
        in_maps.append(
            {
                "xT": np.ascontiguousarray(xs.T),
                "We": We_bf,
                "be": be_bf,
                "Wg": Wg_bf,
                "bg": bg_bf,
            }
        )
    return in_maps, n_tok_pc


def run(x, We, be, Wg, bg, trace=False, **trace_kwargs):
    in_maps, n_tok_pc = make_in_maps(x, We, be, Wg, bg)
    nc = _get_nc(n_tok_pc)
    res = run_bass_kernel_spmd(
        nc, in_maps, core_ids=list(range(N_CORES)), trace=trace, **trace_kwargs
    )
    outs = [res.results[i]["out"] for i in range(N_CORES)]
    return np.concatenate(outs, axis=0), res


def kernel(x, We, be, Wg, bg):
    out, _ = run(x, We, be, Wg, bg, trace=False)
    return out


# revision 2
# speedup vs baseline: 1.0299x; 1.0299x over previous
"""Dense MoE layer (8 experts, all-expert weighted combine) on 8 TRN2 NeuronCores.

Strategy: data-parallel over the token dim. Each core gets a 1024-token shard
(pre-transposed + bf16-cast on host), the full stacked expert weights (bf16),
and computes gate softmax + all 8 expert matmuls + gate-weighted combine
locally. No collectives; host concatenates the 8 output shards.

Per-core device schedule (software-pipelined over token tiles t):
  block t:  y(e=0,t) matmuls | gate logit matmuls(t) | softmax(t) on DVE/ACT |
            PE-transpose of g(t-1) | out(t-1) = y(e=0,t-1)*g0 on DVE |
            bias matmuls g(t-2)@be | out(t-2) += bias psum on DVE
  then for e=1..7, per t: 16 matmuls accumulate y in PSUM f32, one fused DVE
  op: out = psum*g[:,e] + out  (scalar_tensor_tensor, per-partition scalar).

DMA issue is spread across engines (sync: xT+out, scalar: We, gpsimd: small
constants) because each dma_start costs ~0.6us of issue time on its engine.
"""

import os
import sys

import numpy as np

try:
    import concourse.bass as bass  # noqa: F401
except ImportError:  # harness containers stage the repo at /opt/trn_rl_repo
    sys.path.insert(0, "/opt/trn_rl_repo")

from contextlib import ExitStack

import ml_dtypes

import concourse.bass as bass
import concourse.mybir as mybir
import concourse.tile as tile
from concourse import bacc
from concourse.bass_utils import run_bass_kernel_spmd
from concourse.masks import make_identity

N_CORES = 8
N_TOK = 8192
IN_F = 1024
OUT_F = 1024
E = 8
P = 128  # partitions


def build_nc(n_tok_pc: int = N_TOK // N_CORES, debug: bool = False):
    """Build the single-core SPMD Bass program (same program on all 8 cores)."""
    fp32 = mybir.dt.float32
    bf16 = mybir.dt.bfloat16

    K_CH = IN_F // P  # contraction chunks of 128
    T = n_tok_pc // P  # token tiles per core
    assert T >= 2

    nc = bacc.Bacc(
        "TRN2", target_bir_lowering=False, debug=debug, enable_asserts=False
    )

    xT = nc.declare_dram_parameter("xT", [IN_F, n_tok_pc], bf16, isOutput=False)
    We = nc.declare_dram_parameter("We", [E, IN_F, OUT_F], bf16, isOutput=False)
    be = nc.declare_dram_parameter("be", [E, OUT_F], bf16, isOutput=False)
    Wg = nc.declare_dram_parameter("Wg", [P, K_CH, E], bf16, isOutput=False)
    bg = nc.declare_dram_parameter("bg", [1, E], bf16, isOutput=False)
    out = nc.declare_dram_parameter("out", [n_tok_pc, OUT_F], fp32, isOutput=True)

    with tile.TileContext(nc) as tc, ExitStack() as ctx:
        consts = ctx.enter_context(tc.tile_pool(name="consts", bufs=1))
        xpool = ctx.enter_context(tc.tile_pool(name="xpool", bufs=1))
        wepool = ctx.enter_context(tc.tile_pool(name="wepool", bufs=2))
        opool = ctx.enter_context(tc.tile_pool(name="opool", bufs=1))
        gpool = ctx.enter_context(tc.tile_pool(name="gpool", bufs=1))
        small = ctx.enter_context(tc.tile_pool(name="small", bufs=4))
        psum_y = ctx.enter_context(tc.tile_pool(name="psum_y", bufs=3, space="PSUM"))
        psum_g = ctx.enter_context(tc.tile_pool(name="psum_g", bufs=2, space="PSUM"))

        # ---- input DMAs: xT on sync, small constants on gpsimd ----
        xT_sb = []
        for c in range(K_CH):
            xc = xpool.tile([P, n_tok_pc], bf16, tag=f"xt{c}")
            nc.sync.dma_start(out=xc, in_=xT[c * P : (c + 1) * P, :])
            xT_sb.append(xc)

        wg_sb = consts.tile([P, K_CH, E], bf16)
        nc.gpsimd.dma_start(out=wg_sb, in_=Wg[:, :, :])
        bg_sb = consts.tile([1, E], bf16)
        nc.gpsimd.dma_start(out=bg_sb, in_=bg[:, :])
        be_sb = consts.tile([E, OUT_F], bf16)
        nc.gpsimd.dma_start(out=be_sb, in_=be[:, :])

        ident = consts.tile([P, P], fp32)
        make_identity(nc, ident)
        ones_row = consts.tile([1, P], bf16)
        nc.vector.memset(ones_row, 1.0)

        def fetch_we(e):
            tiles = []
            for c in range(K_CH):
                wc = wepool.tile([P, OUT_F], bf16, tag=f"we{c}")
                nc.scalar.dma_start(out=wc, in_=We[e, c * P : (c + 1) * P, :])
                tiles.append(wc)
            return tiles

        # prefetch first two experts
        we_sb = {0: fetch_we(0), 1: fetch_we(1)}

        g_sb = gpool.tile([P, T, E], fp32)
        gT_sb = gpool.tile([E, T, P], bf16)
        out_sb = opool.tile([P, T, OUT_F], fp32)

        def main_mms(e, t):
            py = psum_y.tile([P, OUT_F], fp32, tag="y")
            tok = slice(t * P, (t + 1) * P)
            for c in range(K_CH):
                for h in range(2):
                    hs = slice(h * 512, (h + 1) * 512)
                    nc.tensor.matmul(
                        py[:, hs],
                        lhsT=xT_sb[c][:, tok],
                        rhs=we_sb[e][c][:, hs],
                        start=(c == 0),
                        stop=(c == K_CH - 1),
                    )
            return py

        def gate_softmax(t):
            tok = slice(t * P, (t + 1) * P)
            lg = psum_g.tile([P, E], fp32, tag="g8")
            for c in range(K_CH):
                nc.tensor.matmul(
                    lg,
                    lhsT=xT_sb[c][:, tok],
                    rhs=wg_sb[:, c, :],
                    start=(c == 0),
                    stop=False,
                )
            # += 1 * bg  (K=1 matmul appends the gate bias)
            nc.tensor.matmul(lg, lhsT=ones_row, rhs=bg_sb, start=False, stop=True)

            neg_m = small.tile([P, 1], fp32, tag="negm")
            nc.vector.reduce_max(
                out=neg_m, in_=lg, axis=mybir.AxisListType.X, negate=True
            )
            gexp = small.tile([P, E], fp32, tag="gexp")
            nc.scalar.activation(
                out=gexp,
                in_=lg,
                func=mybir.ActivationFunctionType.Exp,
                bias=neg_m,
                scale=1.0,
            )
            ssum = small.tile([P, 1], fp32, tag="ssum")
            nc.vector.reduce_sum(out=ssum, in_=gexp, axis=mybir.AxisListType.X)
            rsum = small.tile([P, 1], fp32, tag="rsum")
            nc.vector.reciprocal(out=rsum, in_=ssum)
            nc.vector.tensor_scalar_mul(g_sb[:, t, :], gexp, rsum)

        def transpose_g(t):
            gt_ps = psum_g.tile([E, P], fp32, tag="g8")
            nc.tensor.transpose(gt_ps, g_sb[:, t, :], ident)
            nc.scalar.copy(out=gT_sb[:, t, :], in_=gt_ps)

        def combine0(t, py):
            # out[t] = y(e=0) * g[:, 0]   (overwrite-init; bias added later)
            nc.vector.tensor_scalar_mul(
                out_sb[:, t, :], py[:, :], g_sb[:, t, 0:1]
            )

        def bias_mms(t):
            pb = psum_y.tile([P, OUT_F], fp32, tag="y")
            for h in range(2):
                hs = slice(h * 512, (h + 1) * 512)
                nc.tensor.matmul(
                    pb[:, hs], lhsT=gT_sb[:, t, :], rhs=be_sb[:, hs],
                    start=True, stop=True,
                )
            # out[t] += g @ be
            nc.vector.tensor_tensor(
                out=out_sb[:, t, :],
                in0=pb[:, :],
                in1=out_sb[:, t, :],
                op=mybir.AluOpType.add,
            )

        def combine(e, t, py):
            # out[t] = y(e) * g[:, e] + out[t]   (fused on DVE)
            nc.vector.scalar_tensor_tensor(
                out=out_sb[:, t, :],
                in0=py[:, :],
                scalar=g_sb[:, t, e : e + 1],
                in1=out_sb[:, t, :],
                op0=mybir.AluOpType.mult,
                op1=mybir.AluOpType.add,
            )

        # ---- phase A: e=0 pipelined with gate computation ----
        py_live = {}
        for t in range(T):
            py_live[t] = main_mms(0, t)
            gate_softmax(t)
            if t >= 1:
                transpose_g(t - 1)
                combine0(t - 1, py_live.pop(t - 1))
            if t >= 2:
                bias_mms(t - 2)
        transpose_g(T - 1)
        combine0(T - 1, py_live.pop(T - 1))
        bias_mms(T - 2)
        bias_mms(T - 1)

        # ---- phase B: experts 1..7 ----
        for e in range(1, E):
            if e + 1 < E:
                we_sb[e + 1] = fetch_we(e + 1)
            for t in range(T):
                py = main_mms(e, t)
                combine(e, t, py)
            del we_sb[e - 1]

        # ---- write back ----
        for t in range(T):
            nc.sync.dma_start(
                out=out[t * P : (t + 1) * P, :], in_=out_sb[:, t, :]
            )

    nc.compile()
    return nc


_NC_CACHE: dict = {}


def _get_nc(n_tok_pc: int):
    if n_tok_pc not in _NC_CACHE:
        _NC_CACHE[n_tok_pc] = build_nc(n_tok_pc)
    return _NC_CACHE[n_tok_pc]


def make_in_maps(x, We, be, Wg, bg):
    """Host-side sharding: token-shard + transpose x, bf16-cast everything."""
    bf16 = ml_dtypes.bfloat16
    x = np.asarray(x)
    n_tok_pc = x.shape[0] // N_CORES
    We_bf = np.asarray(We).astype(bf16)
    be_bf = np.asarray(be).astype(bf16)
    K_CH = IN_F // P
    # [1024, 8] -> [p, chunk, e]
    Wg_bf = (
        np.asarray(Wg).astype(bf16).reshape(K_CH, P, E).transpose(1, 0, 2).copy()
    )
    bg_bf = np.asarray(bg).astype(bf16).reshape(1, E)
    xbf = x.astype(bf16)
    in_maps = []
    for cid in range(N_CORES):
        xs = xbf[cid * n_tok_pc : (cid + 1) * n_tok_pc]
        in_maps.append(
            {
                "xT": np.ascontiguousarray(xs.T),
                "We": We_bf,
                "be": be_bf,
                "Wg": Wg_bf,
                "bg": bg_bf,
            }
        )
    return in_maps, n_tok_pc


def run(x, We, be, Wg, bg, trace=False, **trace_kwargs):
    in_maps, n_tok_pc = make_in_maps(x, We, be, Wg, bg)
    nc = _get_nc(n_tok_pc)
    res = run_bass_kernel_spmd(
        nc, in_maps, core_ids=list(range(N_CORES)), trace=trace, **trace_kwargs
    )
    outs = [res.results[i]["out"] for i in range(N_CORES)]
    return np.concatenate(outs, axis=0), res


def kernel(x, We, be, Wg, bg):
    out, _ = run(x, We, be, Wg, bg, trace=False)
    return out
